# revision 3
# baseline (speedup 1.0000x reference)
"""Trainium2 Bass kernel for ChamferkNNDist.

Problem (B=8, N=4096, 3-D points):
  chamfer = mean_b mean_j min_i ||adv[b,j] - ori[b,i]||^2
  knn: per adv point, mean of its 5 nearest-neighbour sq-distances within
       adv[b] (excluding self), then a mean+1.05*std threshold mask.
  out = 5*chamfer + 3*knn_loss  (fp32 scalar)

Strategy: data-parallel over batch, one element per core. The device
computes M[j,i] = 2*a_j.x_i - |x_i|^2 (x = adv for kNN, ori for chamfer);
the row-constant -|a_j|^2 is subtracted on the host AFTER the row-max /
top-8 (order-preserving shift), which keeps the large norm terms out of
the matmul's precision budget.

Matmul: fp8 e5m2 DoubleRow (0.5 PE cycles/column - 2x bf16). Operands are
7-level e5m2 splits (3 bits/level): 28 cross-level coordinate terms x 3
dims + 7 norm-level rows paired with exact power-of-2 constants = 91 term
rows, packed [46 partitions x 2 DoubleRow slots]. CPU-validated matrix
error ~2e-5 mean; end-to-end loss rel err ~2.5e-4 (fold-dominated).

PSUM egress (the scan bottleneck; measured per-[128,1024] costs):
  ACT copy PSUM->fp16 SBUF ~580ns | DVE reduce(max) PSUM ~670ns
  DVE TT max fp16 SBUF ~150ns     | DVE MAX8 ~900ns
  kNN:  ACT copies all 4 PSUM quarters to fp16; DVE folds 8:1 with a
        fp16 TT max-tree + one MAX8 over the 512-wide result. adv points
        are pre-sorted by x so folded columns are spatially distant
        (top-6 collisions ~never; CPU-validated).
  cham: 2 quarters DVE-reduced straight from PSUM; 2 quarters ACT-copied,
        fp16-folded, reduced (row max is fold-invariant - exact).
Host finalize (fp64): add na_j back, drop rank-0 (self), mean ranks 1..5,
mean+1.05*std threshold mask, combine with chamfer mean.
"""

import sys

if "/opt/trn_rl_repo" not in sys.path:
    sys.path.insert(0, "/opt/trn_rl_repo")

from contextlib import ExitStack

import numpy as np

import concourse.bacc as bacc
import concourse.tile as tile
from concourse import mybir
from concourse.bass_utils import run_bass_kernel_spmd

F32 = mybir.dt.float32
F16 = mybir.dt.float16
BF16 = mybir.dt.bfloat16
F8 = mybir.dt.float8e5

B = 8
N = 4096
NCORES = 8
Q = 1024  # PSUM quarter width (2 banks); 4 quarters fill all 8 banks

MM = "f8dr"  # "f8dr" | "bf16x2"
KD = 46  # DoubleRow partition count (2*46 slots >= 91 term rows)
L, STEP = 7, 3  # e5m2 split levels / bits per level
SB_BUFS = 3

CHAMFER_W = 5.0
KNN_W = 3.0
KNN_K = 5
KNN_ALPHA = 1.05


def build_program(n=N, reps=1, mm=None):
    mm = mm or MM
    nt = n // 128
    nc = bacc.Bacc("TRN2", target_bir_lowering=False, debug=False)
    if mm == "f8dr":
        ua = nc.dram_tensor("ua", [KD, 2 * n], F8, kind="ExternalInput").ap()
        va = nc.dram_tensor("va", [KD, 2 * n], F8, kind="ExternalInput").ap()
        vo = nc.dram_tensor("vo", [KD, 2 * n], F8, kind="ExternalInput").ap()
    else:
        ua = nc.dram_tensor("ua", [15, n], BF16, kind="ExternalInput").ap()
        va = nc.dram_tensor("va", [15, n], BF16, kind="ExternalInput").ap()
        vo = nc.dram_tensor("vo", [15, n], BF16, kind="ExternalInput").ap()
    top8 = nc.dram_tensor("top8", [n, 8], F16, kind="ExternalOutput").ap()
    cpart = nc.dram_tensor("cpart", [n, 3], F32, kind="ExternalOutput").ap()

    with tile.TileContext(nc) as tc:
        with ExitStack() as ctx:
            const_pool = ctx.enter_context(tc.tile_pool(name="const", bufs=1))
            psum_pool = ctx.enter_context(
                tc.tile_pool(name="ps", bufs=1, space="PSUM")
            )
            sb = ctx.enter_context(tc.tile_pool(name="sb", bufs=SB_BUFS))

            if mm == "f8dr":
                ua_in = const_pool.tile([KD, 2, n], F8)
                nc.sync.dma_start(ua_in[:], ua)
                va_in = const_pool.tile([KD, 2, n], F8)
                nc.sync.dma_start(va_in[:], va)
                vo_in = const_pool.tile([KD, 2, n], F8)
                nc.sync.dma_start(vo_in[:], vo)

                def mm_quarter(dst, t, src, c0):
                    lhsT = ua_in[:, :, t * 128 : (t + 1) * 128]
                    for j in range(Q // 512):
                        nc.tensor.matmul(
                            dst[:, j * 512 : (j + 1) * 512],
                            lhsT,
                            src[:, :, c0 + j * 512 : c0 + j * 512 + 512],
                            start=True,
                            stop=True,
                            perf_mode=mybir.MatmulPerfMode.DoubleRow,
                        )
            else:
                ua_in = const_pool.tile([15, n], BF16)
                nc.sync.dma_start(ua_in[:], ua)
                va_in = const_pool.tile([15, n], BF16)
                nc.sync.dma_start(va_in[:], va)
                vo_in = const_pool.tile([15, n], BF16)
                nc.sync.dma_start(vo_in[:], vo)

                def mm_quarter(dst, t, src, c0):
                    lhsT = ua_in[:, t * 128 : (t + 1) * 128]
                    for j in range(Q // 512):
                        nc.tensor.matmul(
                            dst[:, j * 512 : (j + 1) * 512],
                            lhsT,
                            src[:, c0 + j * 512 : c0 + j * 512 + 512],
                            start=True,
                            stop=True,
                        )

            TAGS = ["pA", "pB", "pC", "pD"]

            def body(_i=None):
                for t in range(nt):
                    # ---- kNN: ACT egress + fp16 fold-tree + MAX8 ----
                    kq = []
                    for q in range(4):
                        p = psum_pool.tile([128, Q], F32, tag=TAGS[q])
                        mm_quarter(p, t, va_in, q * Q)
                        k = sb.tile([128, Q], F16, tag=f"k{q}")
                        nc.scalar.copy(k[:], p[:])
                        kq.append(k)
                    fA = sb.tile([128, Q], F16, tag="fA")
                    nc.vector.tensor_tensor(
                        fA[:], kq[0][:], kq[2][:], op=mybir.AluOpType.max
                    )
                    fB = sb.tile([128, Q], F16, tag="fB")
                    nc.vector.tensor_tensor(
                        fB[:], kq[1][:], kq[3][:], op=mybir.AluOpType.max
                    )
                    kf = sb.tile([128, Q], F16, tag="kf")
                    nc.vector.tensor_tensor(
                        kf[:], fA[:], fB[:], op=mybir.AluOpType.max
                    )
                    kf2 = sb.tile([128, Q // 2], F16, tag="kf2")
                    nc.vector.tensor_tensor(
                        kf2[:], kf[:, : Q // 2], kf[:, Q // 2 :],
                        op=mybir.AluOpType.max,
                    )
                    t8 = sb.tile([128, 8], F16, tag="t8")
                    nc.vector.max(t8[:], kf2[:])

                    # ---- chamfer: 2 direct PSUM reduces + fp16 pair ----
                    r = sb.tile([128, 3], F32, tag="r")
                    cq = []
                    for q in range(4):
                        p = psum_pool.tile([128, Q], F32, tag=TAGS[q])
                        mm_quarter(p, t, vo_in, q * Q)
                        if q in (0, 2):
                            c = sb.tile([128, Q], F16, tag=f"c{q}")
                            nc.scalar.copy(c[:], p[:])
                            cq.append(c)
                        else:
                            nc.vector.tensor_reduce(
                                r[:, (q - 1) // 2 : (q - 1) // 2 + 1],
                                p[:],
                                axis=mybir.AxisListType.X,
                                op=mybir.AluOpType.max,
                            )
                    m = sb.tile([128, Q], F16, tag="m")
                    nc.vector.tensor_tensor(
                        m[:], cq[0][:], cq[1][:], op=mybir.AluOpType.max
                    )
                    m2 = sb.tile([128, Q // 2], F16, tag="m2")
                    nc.vector.tensor_tensor(
                        m2[:], m[:, : Q // 2], m[:, Q // 2 :],
                        op=mybir.AluOpType.max,
                    )
                    nc.vector.tensor_reduce(
                        r[:, 2:3], m2[:],
                        axis=mybir.AxisListType.X, op=mybir.AluOpType.max,
                    )

                    nc.sync.dma_start(top8[t * 128 : (t + 1) * 128, :], t8[:])
                    nc.sync.dma_start(cpart[t * 128 : (t + 1) * 128, :], r[:])

            if reps == 1:
                body()
            else:
                with tc.For_i(0, reps, 1):
                    body()
    nc.compile()
    return nc


def _levels(v, L=L, step=STEP):
    """Multi-level e5m2 split: v ~= sum(levels); level i repr at scale 2^-3i."""
    import ml_dtypes

    f8 = ml_dtypes.float8_e5m2
    out = []
    rem = np.array(v, np.float64, copy=True)
    for i in range(L):
        sc = 2.0 ** (step * i)
        q = (rem * sc).astype(np.float32).astype(f8).astype(np.float64) / sc
        out.append(q)
        rem = rem - q
    return out


def _f8_operands(a_sorted, x, f8):
    """Build (ua_rows, vx_rows) f8 arrays [91, N] so that
    sum_k ua[k,j]*vx[k,i] = 2*a_j.x_i - |x_i|^2 (up to ~2e-5)."""
    s = 0.5
    nx = (x * x).sum(1, dtype=np.float64)
    al = [_levels(2.0 * a_sorted[:, d] * s) for d in range(3)]
    xl = [_levels(x[:, d] / s) for d in range(3)]
    nl = _levels(nx)
    rowsA, rowsB = [], []
    for i in range(L):
        for j in range(L):
            if i + j > L - 1:
                continue
            c = 2.0 ** round(STEP * (i - j) / 2)
            for d in range(3):
                rowsA.append((al[d][i] * c).astype(np.float32))
                rowsB.append((xl[d][j] / c).astype(np.float32))
    for j in range(L):
        c = 2.0 ** (max(0, round(STEP * j / 2)) - 2)
        rowsA.append(np.full(a_sorted.shape[0], -c, np.float32))
        rowsB.append((nl[j] / c).astype(np.float32))
    A = np.stack(rowsA).astype(f8)
    Bm = np.stack(rowsB).astype(f8)
    return A, Bm


def _pack_dr(rows, f8):
    """[91, N] -> [KD, 2, N]: slot s, partition k holds flat row s*KD+k."""
    nrows, n = rows.shape
    out = np.zeros((2 * KD, n), f8)
    out[:nrows] = rows
    return out.reshape(2, KD, n).transpose(1, 0, 2).copy()


def make_inputs(adv_pc, ori_pc, mm=None):
    """Per-core input dicts (+aux na per batch for finalize)."""
    import ml_dtypes

    mm = mm or MM
    adv = np.asarray(adv_pc, dtype=np.float32)
    ori = np.asarray(ori_pc, dtype=np.float32)
    in_maps = []
    aux = []
    for b in range(B):
        a = adv[b]
        a = a[np.argsort(a[:, 0], kind="stable")]
        o = ori[b]
        na = (a * a).sum(1, dtype=np.float64)
        aux.append(na)
        if mm == "f8dr":
            f8 = ml_dtypes.float8_e5m2
            uaA, vaB = _f8_operands(a, a, f8)
            uaA2, voB = _f8_operands(a, o, f8)
            # stationary rows identical for both (built from a only)
            ua_dr = _pack_dr(uaA, f8).reshape(KD, -1)
            va_dr = _pack_dr(vaB, f8).reshape(KD, -1)
            vo_dr = _pack_dr(voB, f8).reshape(KD, -1)
            in_maps.append({"ua": ua_dr, "va": va_dr, "vo": vo_dr})
        else:
            bf = ml_dtypes.bfloat16
            nax = (a * a).sum(1, dtype=np.float32)[None, :]
            no = (o * o).sum(1, dtype=np.float32)[None, :]
            one = np.ones((1, a.shape[0]), np.float32)
            # bf16x2 path computes M = 2a.x - |x|^2 too (norm-free lhsT)
            ua = np.concatenate([2.0 * a.T, -one], 0).astype(np.float32)
            va = np.concatenate([a.T, nax], 0).astype(np.float32)
            vo = np.concatenate([o.T, no], 0).astype(np.float32)

            def split12(mmm, kind):
                hi = mmm.astype(bf)
                lo = (mmm - hi.astype(np.float32)).astype(bf)
                if kind == "u":
                    return np.concatenate([hi, hi, lo], 0)
                return np.concatenate([hi, lo, hi], 0)

            pad = np.zeros((3, a.shape[0]), bf)
            in_maps.append(
                {
                    "ua": np.concatenate([split12(ua, "u"), pad], 0),
                    "va": np.concatenate([split12(va, "v"), pad], 0),
                    "vo": np.concatenate([split12(vo, "v"), pad], 0),
                }
            )
    return in_maps, aux


def finalize(results, aux):
    """Host-side (fp64) final reduction from per-core top8/cpart outputs."""
    loss1 = np.empty(B, np.float64)
    knn = np.empty(B, np.float64)
    for b in range(B):
        na = aux[b]  # [N] |a_j|^2, sorted order
        top8 = results[b]["top8"].astype(np.float64)  # [N, 8] folded M'
        cpart = results[b]["cpart"].astype(np.float64)  # [N, 3] partial maxes
        loss1[b] = (na - cpart.max(axis=1)).mean()
        # -D top8 = M' top8 - na_j; rank 0 is self; ranks 1..5 the 5-NN
        value = na - top8[:, 1 : KNN_K + 1].mean(axis=1)
        mean = value.mean()
        std = value.std(ddof=1)
        thresh = mean + KNN_ALPHA * std
        knn[b] = (value * (value > thresh)).mean()
    total = CHAMFER_W * loss1.mean() + KNN_W * knn.mean()
    return np.float32(total)


_program_cache = {}


def kernel(adv_pc, ori_pc):
    key = MM
    if key not in _program_cache:
        _program_cache[key] = build_program()
    nc = _program_cache[key]
    in_maps, aux = make_inputs(adv_pc, ori_pc)
    res = run_bass_kernel_spmd(nc, in_maps, core_ids=list(range(NCORES)))
    return finalize(res.results, aux)
